# revision 46
# baseline (speedup 1.0000x reference)
"""Trainium2 Bass kernel for nn_AutoregressiveSelfAttention.

Sharding (8 cores): batch (2-way) x head-group (4-way tensor parallel).
Core c: batch c//4, heads [4*(c%4), 4*(c%4)+4).

One-pass attention per core: scores are computed ONCE per (head, sq-tile)
in [sq, sk] layout with q pre-scaled by sqrt(d)=8 and the causal mask
folded in via an identity@cmask matmul accumulation.  The exact row-max
comes from free-axis reduce_max (negate=True) split across DVE and Pool,
and feeds scalar.activation(Exp) directly as a per-partition bias — no
augmented-matmul max folding and no transposed score recompute.  The
exp'd probs are transposed to [sk, sq] with the DMA XBAR
(dma_start_transpose), then the ctx matmul against [v|1] accumulates ctx
and the softmax denominator in one PSUM tile.  Out-proj is row-parallel;
the host sums the 4 head-group partials per batch.
"""
import sys
sys.path.insert(0, "/opt/trn_rl_repo")

import math
import numpy as np

B, S, E, H, D = 2, 2048, 1024, 16, 64
NCORES = 8
HG = 4                  # head-group shards
HPC = H // HG           # 4 heads per core
OC = HPC * D            # 256 per-core projection width
P = 128
NK = E // P             # 8 contraction tiles for projections
NT = S // P             # 16 seq tiles of 128
NJ = S // 512           # 4 seq chunks of 512

_CACHE = {}


def _build():
    import concourse.bacc as bacc
    import concourse.mybir as mybir
    import concourse.tile as tile
    from concourse.masks import make_identity, make_causal_mask

    dt = mybir.dt
    f32, f16 = dt.float32, dt.float16
    AX = mybir.AxisListType.X

    nc = bacc.Bacc(None, target_bir_lowering=False, debug=False)
    with tile.TileContext(nc) as tc:
        with tc.tile_pool(name="dram", bufs=1, space="DRAM") as dram, \
             tc.tile_pool(name="persist", bufs=1) as pers, \
             tc.tile_pool(name="strm", bufs=3) as strm, \
             tc.tile_pool(name="tmp", bufs=8) as tmp, \
             tc.tile_pool(name="ps", bufs=1, space="PSUM") as ps:

            # ---- DRAM I/O ----
            xt = dram.tile([E, S], f16, kind="ExternalInput", name="xt", uniquify=False)
            wq = dram.tile([E, OC], f16, kind="ExternalInput", name="wq", uniquify=False)
            wk = dram.tile([E, OC], f16, kind="ExternalInput", name="wk", uniquify=False)
            wv = dram.tile([E, OC], f16, kind="ExternalInput", name="wv", uniquify=False)
            wo = dram.tile([OC, E], f16, kind="ExternalInput", name="wo", uniquify=False)
            outT = dram.tile([E, S], dt.bfloat16, kind="ExternalOutput", name="outT", uniquify=False)

            # ---- persistent SBUF ----
            xt_sb = pers.tile([P, NK, S], f16)
            wq_sb = pers.tile([P, NK, OC], f16)
            wk_sb = pers.tile([P, NK, OC], f16)
            wv_sb = pers.tile([P, NK, OC], f16)
            wo_sb = pers.tile([P, 2, E], f16)
            qp_sb = pers.tile([P, 2, S], f16)       # 8*qT, head-pair stacked
            kp_sb = pers.tile([P, 2, S], f16)       # kT, head-pair stacked
            vv0 = pers.tile([P, NT, 2, 65], f16)    # heads 0,2: [v(0:64) | ones]
            vv1 = pers.tile([P, NT, 2, P], f16)     # heads 1,3: [ones|0*63|v(64:128)]
            ctxn = pers.tile([P, 2, S], f16)        # normalized ctx, pair stacked
            ident16 = pers.tile([P, P], f16)
            cmask16 = pers.tile([P, P], f16)        # 0 / -30000 causal block

            # ---- input DMAs ----
            xt_v = xt[:].rearrange("(k p) s -> p k s", p=P)
            wq_v = wq[:].rearrange("(k p) o -> p k o", p=P)
            wk_v = wk[:].rearrange("(k p) o -> p k o", p=P)
            wv_v = wv[:].rearrange("(k p) o -> p k o", p=P)
            wo_v = wo[:].rearrange("(k p) e -> p k e", p=P)
            outT_v = outT[:].rearrange("(o p) s -> p o s", p=P)
            # batched input loads: few DMA instructions, fewer round-robin
            # HWDGE-sem lane collisions with the spine transposes later
            # column-chunked xt loads: q/k projection of seq-chunk 0 only
            # needs xt[:, 0:512] (all contraction rows), so the attention
            # spine starts after ~1/4 of the xt bytes have landed
            nc.sync.dma_start(out=xt_sb[:, :, 0:512], in_=xt_v[:, :, 0:512])
            nc.sync.dma_start(out=wq_sb[:, :, :], in_=wq_v)
            nc.sync.dma_start(out=wk_sb[:, :, :], in_=wk_v)
            nc.sync.dma_start(out=wv_sb[:, :, :], in_=wv_v)
            nc.sync.dma_start(out=xt_sb[:, :, 512:1024],
                              in_=xt_v[:, :, 512:1024])
            nc.sync.dma_start(out=xt_sb[:, :, 1024:2048],
                              in_=xt_v[:, :, 1024:2048])
            nc.sync.dma_start(out=wo_sb[:, :, :], in_=wo_v)

            # ---- constants ----
            make_identity(nc, ident16[:, :])
            make_causal_mask(nc, cmask16[:, :], mask_val=-30000.0)
            nc.gpsimd.memset(vv0[:, :, :, 64:65], 1.0)
            nc.gpsimd.memset(vv1[:, :, :, 0:1], 1.0)
            nc.gpsimd.memset(vv1[:, :, :, 1:64], 0.0)

            # ---- q/k projections (transposed layout, pair-stacked) ----
            # q is scaled by 8 (sqrt(d), faithful to the source's inverted
            # scale) so the exp bias is just -rowmax of the scaled scores.
            # Emitted per seq-chunk so attention on chunk 0 can start early.
            def emit_qkproj(j, which):
                dst, w_sb, scl = ((qp_sb, wq_sb, 8.0), (kp_sb, wk_sb, None))[which]
                for ot in range(2):
                    pp = ps.tile([P, 512], f32, tag="s", bufs=7)
                    for k in range(NK):
                        nc.tensor.matmul(
                            pp[:, :],
                            w_sb[:, k, 128 * ot:128 * ot + 128],
                            xt_sb[:, k, 512 * j:512 * j + 512],
                            start=(k == 0), stop=(k == NK - 1))
                    d_ap = dst[:, ot, 512 * j:512 * j + 512]
                    if scl is None:
                        nc.scalar.copy(d_ap, pp[:, :])
                    else:
                        nc.scalar.mul(d_ap, pp[:, :], scl)

            def emit_vproj(st):
                pv = ps.tile([P, OC], f32, tag="s", bufs=7)
                for k in range(NK):
                    nc.tensor.matmul(
                        pv[:, :], xt_sb[:, k, P * st:P * st + P], wv_sb[:, k, :],
                        start=(k == 0), stop=(k == NK - 1))
                pv4 = pv[:, :].rearrange("p (g x d) -> p g x d", g=2, x=2)
                nc.scalar.copy(vv0[:, st, :, 0:64], pv4[:, :, 0, :])
                nc.scalar.copy(vv1[:, st, :, 64:P], pv4[:, :, 1, :])

            def emit_scores(pr, hh, t, j, ptile, pt_buf):
                # scores8 [sq, sk] for one sq-tile of one head, causal mask
                # folded in; exact rowmax -> exp bias; exp'd probs -> SBUF;
                # XBAR-transpose into pt_buf block-stacked [sk, blk, sq].
                # Score PSUM tiles are 2-bank (1024 f32) so the reduce and exp
                # run as few wide instructions instead of many 512 ones.
                ncols = (t + 1) * P
                nch = (ncols + 511) // 512
                m4 = tmp.tile([P, 4], f32, tag="m4")
                nm8 = tmp.tile([P, 1], f32, tag="nm8")
                tpos = (0, 0) if hh == 0 else (64, 0)
                sps = []
                for c in range(nch):
                    n = min(512, ncols - 512 * c)
                    sp = ps.tile([P, 512], f32, tag="s", bufs=7)
                    sps.append((sp, n))
                    last = c == nch - 1
                    nc.tensor.matmul(
                        sp[:, :n], qp_sb[64 * hh:64 * hh + 64, pr, P * t:P * t + P],
                        kp_sb[64 * hh:64 * hh + 64, pr, 512 * c:512 * c + n],
                        start=True, stop=not last, tile_position=tpos)
                    if last:
                        nc.tensor.matmul(sp[:, n - P:n], ident16[:, :],
                                         cmask16[:, :], start=False, stop=True)
                    nc.vector.reduce_max(m4[:, c:c + 1], sp[:, :n], axis=AX)
                nc.vector.reduce_max(nm8[:, 0:1], m4[:, 0:nch], axis=AX,
                                     negate=True)
                for c, (sp, n) in enumerate(sps):
                    nc.scalar.activation(ptile[:, 512 * c:512 * c + n], sp[:, :n],
                                         mybir.ActivationFunctionType.Exp,
                                         bias=nm8[:, 0:1], scale=1.0)
                # XBAR transpose on the SP hwdge queue (kept free of other
                # DMAs so spine transposes never queue behind unrelated waits).
                # Both APs must be CONTIGUOUS: a strided destination silently
                # produces wrong output on hardware (see tile_matmul.py).
                # Big tiles transpose in two pieces so the first piece starts
                # while the tail chunks are still being exp'd.
                tm = t - 4 * j
                if nch >= 2:
                    split = (nch - 1) * 512
                    nc.sync.dma_start(
                        out=pt_buf[:, tm, 0:split // P, :],
                        in_=ptile[:, 0:split], transpose=True)
                    nc.sync.dma_start(
                        out=pt_buf[:, tm, split // P:t + 1, :],
                        in_=ptile[:, split:ncols], transpose=True)
                else:
                    nc.sync.dma_start(
                        out=pt_buf[:, tm, 0:t + 1, :],
                        in_=ptile[:, 0:ncols], transpose=True)

            def emit_ctx(pr, hh, j, pt_buf):
                # ctx accumulation over sk blocks + rowsum, then normalize.
                nb = 4 * j + 4
                ctxp = ps.tile([P, 512], f32, tag="ctx", bufs=1)
                for c in range(nb):
                    tm0 = max(0, c - 4 * j)
                    soff = P * tm0
                    n = 512 - soff
                    lhsT = vv0[:, c, pr, :] if hh == 0 else vv1[:, c, pr, :]
                    nc.tensor.matmul(
                        ctxp[0:(65 if hh == 0 else P), soff:soff + n],
                        lhsT, pt_buf[:, tm0:4, c, :],
                        start=(c == 0), stop=(c == nb - 1))
                rsrow = 64 if hh == 0 else 0
                rr = tmp.tile([P, 512], f32, tag="rr", bufs=2)
                nc.vector.reciprocal(rr[0:1, :], ctxp[rsrow:rsrow + 1, :])
                rb = tmp.tile([P, 512], f32, tag="rb", bufs=2)
                nc.gpsimd.partition_broadcast(
                    rb[0:64, :], rr[0:1, :], channels=64)
                nc.vector.tensor_mul(
                    ctxn[64 * hh:64 * hh + 64, pr, 512 * j:512 * j + 512],
                    ctxp[64 * hh:64 * hh + 64, :],
                    rb[0:64, :])

            def emit_outproj(j):
                ob = strm.tile([P, 8, 512], dt.bfloat16, tag="ob", bufs=2)
                for oo in range(E // P):
                    po = ps.tile([P, 512], f32, tag="s", bufs=7)
                    for kt in range(2):
                        nc.tensor.matmul(
                            po[:, :], wo_sb[:, kt, P * oo:P * oo + P],
                            ctxn[:, kt, 512 * j:512 * j + 512],
                            start=(kt == 0), stop=(kt == 1))
                    if oo % 2 == 0:
                        nc.vector.tensor_copy(ob[:, oo, :], po[:, :])
                    else:
                        nc.scalar.copy(ob[:, oo, :], po[:, :])
                # two batched bf16 stores per chunk (second half can go
                # as soon as its four copies land)
                nc.scalar.dma_start(out=outT_v[:, 0:4, 512 * j:512 * j + 512],
                                    in_=ob[:, 0:4, :])
                nc.scalar.dma_start(out=outT_v[:, 4:8, 512 * j:512 * j + 512],
                                    in_=ob[:, 4:8, :])

            # ---- attention pipeline (software-pipelined emission) ----
            # ctx of a stream is emitted after the NEXT stream's scores so the
            # in-order PE queue never waits on the reduce/exp/transpose chain;
            # outproj of chunk j lands inside chunk j+1 for the same reason.
            emit_qkproj(0, 0)
            emit_qkproj(0, 1)
            prev_ctx = None
            for j in range(NJ):
                for st in range(4 * j, 4 * j + 2):
                    emit_vproj(st)
                for si, (pr, hh) in enumerate(((0, 0), (0, 1), (1, 0), (1, 1))):
                    pt_buf = strm.tile([P, 4, NT, P], f16, tag="pt", bufs=3)
                    for t in range(4 * j, 4 * j + 4):
                        ptile = strm.tile([P, (t + 1) * P], f16, tag="p", bufs=8)
                        emit_scores(pr, hh, t, j, ptile, pt_buf)
                    if prev_ctx is not None:
                        emit_ctx(*prev_ctx)
                    prev_ctx = (pr, hh, j, pt_buf)
                    # PE/Pool filler between streams: rest of this chunk's
                    # v-projection, then next chunk's q/k projections
                    if si == 0:
                        for st in range(4 * j + 2, 4 * j + 4):
                            emit_vproj(st)
                    elif si == 1 and j + 1 < NJ:
                        emit_qkproj(j + 1, 0)
                    elif si == 2 and j + 1 < NJ:
                        emit_qkproj(j + 1, 1)
                if j > 0:
                    emit_outproj(j - 1)
            emit_ctx(*prev_ctx)
            emit_outproj(NJ - 1)

    nc.compile()
    return nc


def _get_nc():
    if "nc" not in _CACHE:
        _CACHE["nc"] = _build()
    return _CACHE["nc"]


def _make_cached_runner(nc):
    """Trace/compile the 8-core PJRT executable once; reuse on later calls.

    Mirrors concourse.bass2jax.run_bass_via_pjrt's multi-core branch, but
    keeps the jitted shard_map so repeat kernel() calls skip re-trace and
    re-lowering (the NEFF itself is already cached by neuronx_cc_hook).
    """
    import jax
    import jax.numpy as jnp
    from jax.sharding import Mesh, PartitionSpec
    from jax.experimental.shard_map import shard_map
    from concourse import bass2jax, mybir

    bass2jax.install_neuronx_cc_hook()
    partition_name = nc.partition_id_tensor.name if nc.partition_id_tensor else None
    in_names, out_names, out_avals = [], [], []
    for alloc in nc.m.functions[0].allocations:
        if not isinstance(alloc, mybir.MemoryLocationSet):
            continue
        name = alloc.memorylocations[0].name
        if alloc.kind == "ExternalInput":
            if name != partition_name:
                in_names.append(name)
        elif alloc.kind == "ExternalOutput":
            out_names.append(name)
            out_avals.append(jax.core.ShapedArray(
                tuple(alloc.tensor_shape), mybir.dt.np(alloc.dtype)))
    n_params = len(in_names)
    n_outs = len(out_avals)
    all_names = list(in_names) + list(out_names)
    if partition_name is not None:
        all_names.append(partition_name)

    def _body(*args):
        operands = list(args)
        if partition_name is not None:
            operands.append(bass2jax.partition_id_tensor())
        outs = bass2jax._bass_exec_p.bind(
            *operands,
            out_avals=tuple(out_avals),
            in_names=tuple(all_names),
            out_names=tuple(out_names),
            lowering_input_output_aliases=(),
            sim_require_finite=True,
            sim_require_nnan=True,
            nc=nc,
        )
        return tuple(outs)

    devices = jax.devices()[:NCORES]
    mesh = Mesh(np.asarray(devices), ("core",))
    in_specs = (PartitionSpec("core"),) * (n_params + n_outs)
    out_specs = (PartitionSpec("core"),) * n_outs
    sharded = jax.jit(
        shard_map(_body, mesh=mesh, in_specs=in_specs, out_specs=out_specs,
                  check_rep=False),
        donate_argnums=tuple(range(n_params, n_params + n_outs)),
        keep_unused=True)

    def run(in_maps):
        concat_in = [
            np.concatenate([np.asarray(in_maps[c][nm]) for c in range(NCORES)],
                           axis=0)
            for nm in in_names]
        concat_zeros = [
            np.zeros((NCORES * a.shape[0], *a.shape[1:]), a.dtype)
            for a in out_avals]
        out_arrs = sharded(*concat_in, *concat_zeros)
        return [
            {nm: np.asarray(out_arrs[i]).reshape(NCORES, *out_avals[i].shape)[c]
             for i, nm in enumerate(out_names)}
            for c in range(NCORES)]

    return run


def kernel(x, Wq, Wk, Wv, Wo):
    from concourse.bass_utils import run_bass_kernel_spmd

    # Force host numpy immediately: if the caller hands us jax arrays, any
    # .astype/.T on them would dispatch tiny jit programs to the neuron
    # backend, which wedges the device (known neuron-jit crash path).
    x, Wq, Wk, Wv, Wo = (np.asarray(a) for a in (x, Wq, Wk, Wv, Wo))

    nc = _get_nc()
    x16 = np.ascontiguousarray(x.astype(np.float16))
    Wq16 = Wq.astype(np.float16)
    Wk16 = Wk.astype(np.float16)
    Wv16 = Wv.astype(np.float16)
    Wo16 = Wo.astype(np.float16)

    xTs = [np.ascontiguousarray(x16[b].T) for b in range(B)]
    in_maps = []
    for c in range(NCORES):
        b, hg = divmod(c, HG)
        hsl = slice(OC * hg, OC * hg + OC)
        in_maps.append({
            "xt": xTs[b],
            "wq": np.ascontiguousarray(Wq16[hsl, :].T),
            "wk": np.ascontiguousarray(Wk16[hsl, :].T),
            "wv": np.ascontiguousarray(Wv16[hsl, :].T),
            "wo": np.ascontiguousarray(Wo16[:, hsl].T),
        })

    if "runner" in _CACHE:
        results = _CACHE["runner"](in_maps)
    else:
        # first call: compile + run through the sanctioned entry point,
        # then build the cached executable for subsequent calls
        results = run_bass_kernel_spmd(nc, in_maps, list(range(NCORES))).results
        _CACHE["runner"] = _make_cached_runner(nc)

    out = np.zeros((B, S, E), np.float32)
    for c in range(NCORES):
        b = c // HG
        out[b] += results[c]["outT"].T.astype(np.float32)
    return out
